# revision 2
# baseline (speedup 1.0000x reference)
"""GNN PNA Trainium2 Bass kernel V2: fp16 pipeline, SBUF-resident ea/h/stats,
chunk-major h layout with chunked allgather, per-column indirect gathers.

Self-contained: hardcodes problem shapes. kernel(**inputs) -> [4000, 1] f32.
"""
import sys
sys.path.insert(0, "/opt/trn_rl_repo")
import numpy as np

import concourse.bass as bass
import concourse.bacc as bacc
import concourse.tile as tile
from concourse import mybir
from concourse.bass_utils import run_bass_kernel_spmd
from concourse.masks import make_identity

fp32 = mybir.dt.float32
fp16 = mybir.dt.float16
i32 = mybir.dt.int32
AF = mybir.ActivationFunctionType
OP = mybir.AluOpType

N, E, G, D = 100000, 400000, 4000, 70
NC = 8
ATOM_DIMS = np.array([119, 5, 12, 12, 10, 6, 6, 2, 2])
ATOM_OFFSETS = np.concatenate([[0], np.cumsum(ATOM_DIMS)[:-1]]).astype(np.int64)
DEG_HIST = np.array([0.0, 100.0, 400.0, 300.0, 200.0])
_bins = np.arange(len(DEG_HIST), dtype=np.float64)
AVG_LOG = float((np.log(_bins + 1.0) * DEG_HIST).sum() / DEG_HIST.sum())
BN_EPS = 1e-5
STD_EPS = 1e-5
P = 128
BT = 4
EA_PAD = -30000.0     # fp16-safe pad (message -> 0 after relu)
BIG = 30000.0


def _insert_axis(ap_obj, pos, count):
    lst = [list(x) for x in ap_obj.ap]
    lst = lst[:pos] + [[0, count]] + lst[pos:]
    return bass.AP(ap_obj.tensor, ap_obj.offset, lst)


def _prep(x, edge_index, edge_attr, batch, atom_emb):
    src = np.asarray(edge_index[0], np.int64)
    dst = np.asarray(edge_index[1], np.int64)
    batch = np.asarray(batch, np.int64)
    ea = np.asarray(edge_attr, np.float32)

    deg = np.bincount(dst, minlength=N)
    eorder = np.argsort(dst, kind="stable")
    rowptr = np.zeros(N + 1, np.int64)
    rowptr[1:] = np.cumsum(deg)

    gcnt = np.bincount(batch, minlength=G)
    gnode_start = np.zeros(G + 1, np.int64)
    gnode_start[1:] = np.cumsum(gcnt)

    dmax = int(deg.max())
    exact_ds = list(range(0, min(dmax, 8) + 1))
    has_tail = dmax > 8
    dtail = dmax if has_tail else 0
    ngroups0 = len(exact_ds) + (1 if has_tail else 0)

    # per-graph degree-group histograms, then greedy balance across cores
    node_grp = np.where(deg <= 8, deg, 9)
    ghist = np.zeros((G, ngroups0), np.int64)
    for g in range(ngroups0):
        ghist[:, g] = np.bincount(batch[node_grp == g], minlength=G)
    wg = np.array([max(d, 1) for d in exact_ds] + ([dtail] if has_tail else []), np.float64)
    order = np.argsort(-gcnt, kind="stable")
    counts = np.zeros((NC, ngroups0), np.float64)
    tot = np.zeros(NC, np.float64)
    core_of_graph = np.zeros(G, np.int64)
    for gid in order:
        h = ghist[gid]
        best, bestc = None, 0
        for c in range(NC):
            sc = ((counts[c] + h) ** 2 * wg).sum() + 4.0 * (tot[c] + gcnt[gid]) ** 2
            if best is None or sc < best:
                best, bestc = sc, c
        core_of_graph[gid] = bestc
        counts[bestc] += h
        tot[bestc] += gcnt[gid]
    core_graph_list = [np.nonzero(core_of_graph == c)[0] for c in range(NC)]

    core_group_nodes = []
    for c in range(NC):
        nids = np.concatenate([np.arange(gnode_start[gg], gnode_start[gg + 1])
                               for gg in core_graph_list[c]]) if len(core_graph_list[c]) else np.zeros(0, np.int64)
        nd = deg[nids]
        groups = [nids[nd == d] for d in exact_ds]
        if has_tail:
            tl = nids[nd >= 9]
            groups.append(tl[np.argsort(-deg[tl], kind="stable")])  # tail sorted by degree desc
        core_group_nodes.append(groups)

    ngroups = len(exact_ds) + (1 if has_tail else 0)
    dvals = exact_ds + ([dtail] if has_tail else [])
    NT_g = [max((len(core_group_nodes[c][g]) + P - 1) // P for c in range(NC)) for g in range(ngroups)]
    NT = sum(NT_g)
    NB = NT * P

    proc = np.full((NC, NB), -1, np.int64)
    ti = 0
    goff = []
    for g in range(ngroups):
        goff.append(ti)
        ti += NT_g[g]
    for c in range(NC):
        for g in range(ngroups):
            nodes = core_group_nodes[c][g]
            off = goff[g] * P
            proc[c, off:off + len(nodes)] = nodes

    # per-tile degree for the tail group: max real degree in tile across cores
    tail_tile_d = {}
    if has_tail:
        gt = ngroups - 1
        for ti_loc in range(NT_g[gt]):
            m = 1
            for c in range(NC):
                nodes = core_group_nodes[c][gt]
                if ti_loc * P < len(nodes):
                    m = max(m, int(deg[nodes[ti_loc * P]]))
            tail_tile_d[ti_loc] = m

    # blocks: (g, d, t0, nb) with nb capped so nb*d <= 32; tail uses per-block max d
    blocks = []
    for g in range(ngroups):
        d = dvals[g]
        is_tl = has_tail and g == ngroups - 1
        if is_tl:
            b0 = 0
            while b0 < NT_g[g]:
                dblk = tail_tile_d[b0]
                step = max(1, min(BT, 32 // max(dblk, 1)))
                nb = min(step, NT_g[g] - b0)
                dblk = max(tail_tile_d[b0 + i] for i in range(nb))
                blocks.append((g, dblk, goff[g] + b0, nb))
                b0 += nb
        else:
            step = BT if d == 0 else max(1, min(BT, 32 // max(d, 1)))
            for b0 in range(0, NT_g[g], step):
                nb = min(step, NT_g[g] - b0)
                blocks.append((g, d, goff[g] + b0, nb))

    # column layout: assign col offsets to msg blocks in block order
    msg_blocks = []
    col = 0
    bcol = 0
    for (g, d, t0, nb) in blocks:
        if d == 0:
            continue
        is_tail = has_tail and g == ngroups - 1
        msg_blocks.append((g, d, t0, nb, col, bcol))
        col += nb * d
        if is_tail:
            bcol += nb * d
    NIDX = col
    bigpad_cols = max(bcol, 1)

    # Tsplit: block boundary with cumulative gather cols ~72%
    cum = 0
    Tsplit = NT
    for (g, d, t0, nb, col0, _b) in msg_blocks:
        cum += nb * d
        if cum >= 0.85 * NIDX:
            Tsplit = t0 + nb
            break
    Tsplit = max(1, min(Tsplit, NT - 1))
    NT0, NT1 = Tsplit, NT - Tsplit
    NPAD = NC * P * NT

    # chunk-major gpos: node at (c, t, p)
    ii = np.arange(NB)
    tt, pp = ii // P, ii % P
    rowmap = np.where(tt < NT0,
                      pp * NT0 + tt,
                      NC * P * NT0 + pp * NT1 + (tt - NT0))  # without core offset terms
    gpos_of_node = np.full(N, -1, np.int64)
    pos4_of_node = np.full(N, -1, np.int64)   # layer-4 local rows p*NT + t
    for c in range(NC):
        mask = proc[c] >= 0
        base = np.where(tt < NT0, c * P * NT0, c * P * NT1)
        gpos_of_node[proc[c][mask]] = (base + rowmap)[mask]
        pos4_of_node[proc[c][mask]] = (pp * NT + tt)[mask]
    assert (gpos_of_node >= 0).sum() == N

    # srcidx + ea (fp16) + bigpad per core
    srcidx = np.full((NC, P, max(NIDX, 1)), NPAD, np.int32)
    eaflat = np.zeros((NC, P, max(NIDX, 1) * D), np.float16)
    bigpad = np.zeros((NC, P, bigpad_cols), np.float16)
    ea16 = ea.astype(np.float16)
    for c in range(NC):
        for (g, d, t0, nb, col0, bc0) in msg_blocks:
            is_tail = has_tail and g == ngroups - 1
            blk = np.full((P, nb * d, D), EA_PAD, np.float16)
            for i in range(nb):
                t = t0 + i
                for p in range(P):
                    node = proc[c, t * P + p]
                    if node < 0:
                        continue
                    nd = int(deg[node])
                    use = min(nd, d)
                    eids = eorder[rowptr[node]:rowptr[node] + use]
                    blk[p, i * d:i * d + use] = ea16[eids]
                    srcidx[c, p, col0 + i * d:col0 + i * d + use] = gpos_of_node[src[eids]]
                    if is_tail and nd < d:
                        bigpad[c, p, bc0 + i * d + nd:bc0 + (i + 1) * d] = BIG
            eaflat[c, :, col0 * D:(col0 + nb * d) * D] = blk.reshape(P, -1)

    # expanded stats [NC, P, NT*D] fp16
    invcexp = np.zeros((NC, P, NT * D), np.float16)
    ampexp = np.zeros((NC, P, NT * D), np.float16)
    iampexp = np.zeros((NC, P, NT * D), np.float16)
    for c in range(NC):
        nodes = proc[c]
        dd = np.where(nodes >= 0, deg[np.clip(nodes, 0, N - 1)], 0).astype(np.float64)
        dsafe = np.maximum(dd, 1.0)
        amp = np.log(dsafe + 1.0) / AVG_LOG
        st = np.stack([1.0 / dsafe, amp, 1.0 / amp])          # [3, NB]
        st = st.reshape(3, NT, P).transpose(0, 2, 1)          # [3, P, NT]
        ex = np.repeat(st[:, :, :, None], D, axis=3).reshape(3, P, NT * D).astype(np.float16)
        invcexp[c], ampexp[c], iampexp[c] = ex[0], ex[1], ex[2]

    # h0 in chunk-major full layout + per-core [P, NT*D]
    xl = np.asarray(x, np.int64)
    emb = np.asarray(atom_emb, np.float32)
    h0_all = emb[xl + ATOM_OFFSETS[None, :]].sum(axis=1).astype(np.float16)  # [N, D]
    h0_full = np.zeros((NPAD + 1, D), np.float16)
    h0_own = np.zeros((NC, P, NT * D), np.float16)
    for c in range(NC):
        mask = proc[c] >= 0
        nodes = proc[c][mask]
        base = np.where(tt < NT0, c * P * NT0, c * P * NT1)
        h0_full[(base + rowmap)[mask]] = h0_all[nodes]
        pm, tm = pp[mask], tt[mask]
        h0_own[c].reshape(P, NT, D)[pm, tm] = h0_all[nodes]

    # pooling: per core graphs sorted by size desc; indirect gather columns
    core_graphs = []
    for c in range(NC):
        gids = core_graph_list[c]
        order2 = np.argsort(-gcnt[gids], kind="stable")
        core_graphs.append(gids[order2])
    NGT = max((len(cg) + P - 1) // P for cg in core_graphs)
    KG_t = []
    for t in range(NGT):
        m = 1
        for c in range(NC):
            cg = core_graphs[c]
            if t * P < len(cg):
                m = max(m, int(gcnt[cg[t * P]]))
        KG_t.append(m)
    npoolcols = sum(KG_t)
    poolidx = np.full((NC, P, npoolcols), P * NT, np.int32)
    ginvT = np.ones((NC, P, NGT), np.float32)
    pc = 0
    pool_cols = []
    for t in range(NGT):
        pool_cols.append(pc)
        for c in range(NC):
            cg = core_graphs[c]
            for p in range(P):
                if t * P + p >= len(cg):
                    continue
                gid = cg[t * P + p]
                sz = int(gcnt[gid])
                ginvT[c, p, t] = 1.0 / max(sz, 1)
                if sz > 0:
                    nids = np.arange(gnode_start[gid], gnode_start[gid] + sz)
                    poolidx[c, p, pc:pc + sz] = pos4_of_node[nids].astype(np.int32)
        pc += KG_t[t]

    cfg = dict(NT=NT, NT0=NT0, NT1=NT1, NPAD=NPAD, NIDX=NIDX,
               blocks=blocks, msg_blocks=msg_blocks, dvals=dvals, goff=goff,
               ngroups=ngroups, has_tail=has_tail, dtail=dtail,
               NGT=NGT, KG_t=KG_t, pool_cols=pool_cols, npoolcols=npoolcols,
               bigpad_cols=bigpad_cols)
    arrays = dict(srcidx=srcidx, eaflat=eaflat, bigpad=bigpad,
                  invcexp=invcexp, ampexp=ampexp, iampexp=iampexp,
                  h0_full=h0_full, h0_own=h0_own, poolidx=poolidx, ginvT=ginvT)
    asm = dict(core_graphs=core_graphs)
    return cfg, arrays, asm


def _prep_weights(post_w, post_b, bn_gamma, bn_beta, mlp_w1, mlp_b1, mlp_w2, mlp_b2, mlp_w3, mlp_b3):
    post_w = np.asarray(post_w, np.float32)
    post_b = np.asarray(post_b, np.float32)
    bn_gamma = np.asarray(bn_gamma, np.float32)
    bn_beta = np.asarray(bn_beta, np.float32)
    inv_std_bn = np.float32(1.0 / np.sqrt(1.0 + BN_EPS))
    Grep = bn_gamma * inv_std_bn                      # [4, 70]
    wch = np.zeros((4, 3, P, 210), np.float16)
    for l in range(4):
        for ch in range(3):
            r0, r1 = ch * 128, min((ch + 1) * 128, 280)
            rows = r1 - r0
            for s in range(3):
                wch[l, ch, :rows, s * 70:(s + 1) * 70] = (
                    post_w[l, s * 280 + r0:s * 280 + r1, :] * Grep[l][None, :]
                ).astype(np.float16)
    B2 = post_b * Grep + bn_beta                       # [4, 70]
    b2r = np.broadcast_to(B2.reshape(1, 4 * 70), (P, 4 * 70)).astype(np.float16).copy()
    mlpb = np.concatenate([np.asarray(mlp_b1, np.float32), np.asarray(mlp_b2, np.float32),
                           np.asarray(mlp_b3, np.float32)])
    mlpb = np.broadcast_to(mlpb, (P, mlpb.size)).copy()
    return dict(wch=wch, b2r=b2r, mlpb=mlpb,
                w1=np.asarray(mlp_w1, np.float32), w2=np.asarray(mlp_w2, np.float32),
                w3=np.asarray(mlp_w3, np.float32))


def _fold_minmax(nc, spool, g4, d, nb, out_slice, op, tag):
    k = d
    cur = g4
    first = True
    while k > 1:
        h = (k + 1) // 2
        if k == 2:
            nc.vector.tensor_tensor(
                out=out_slice,
                in0=cur[:, :, 0:1].rearrange("p t j f -> p t (j f)"),
                in1=cur[:, :, 1:2].rearrange("p t j f -> p t (j f)"), op=op)
            return
        if first:
            scr = spool.tile([g4.shape[0], nb * h * D], fp16, tag=tag)
            scr3 = scr[:].rearrange("p (t j f) -> p t j f", t=nb, j=h)
            nc.vector.tensor_tensor(out=scr3[:, :, 0:h], in0=cur[:, :, 0:h], in1=cur[:, :, k - h:k], op=op)
            cur = scr3
            first = False
        else:
            nc.vector.tensor_tensor(out=cur[:, :, 0:h], in0=cur[:, :, 0:h], in1=cur[:, :, k - h:k], op=op)
        k = h


def _fold_sum(nc, spool, g4, d, nb, out_slice, tag):
    k = d
    cur = g4
    first = True
    while k > 1:
        h = k // 2
        rem = k - h
        if k == 2:
            nc.vector.tensor_tensor(
                out=out_slice,
                in0=cur[:, :, 0:1].rearrange("p t j f -> p t (j f)"),
                in1=cur[:, :, 1:2].rearrange("p t j f -> p t (j f)"), op=OP.add)
            return
        if first:
            scr = spool.tile([g4.shape[0], nb * rem * D], fp16, tag=tag)
            scr3 = scr[:].rearrange("p (t j f) -> p t j f", t=nb, j=rem)
            nc.vector.tensor_tensor(out=scr3[:, :, 0:h], in0=cur[:, :, 0:h], in1=cur[:, :, k - h:k], op=OP.add)
            if k % 2 == 1:
                nc.vector.tensor_copy(out=scr3[:, :, h:h + 1], in_=cur[:, :, h:h + 1])
            cur = scr3
            first = False
        else:
            nc.vector.tensor_tensor(out=cur[:, :, 0:h], in0=cur[:, :, 0:h], in1=cur[:, :, k - h:k], op=OP.add)
        k = rem


def _build(cfg):
    NT, NT0, NT1, NPAD, NIDX = cfg["NT"], cfg["NT0"], cfg["NT1"], cfg["NPAD"], cfg["NIDX"]
    NGT, npoolcols = cfg["NGT"], cfg["npoolcols"]

    nc = bacc.Bacc("TRN2", target_bir_lowering=False, debug=False, num_devices=NC)
    # inputs
    h0_own_t = nc.dram_tensor("h0_own", [P, NT * D], fp16, kind="ExternalInput").ap()
    h0_full_t = nc.dram_tensor("h0_full", [NPAD + 1, D], fp16, kind="ExternalInput").ap()
    srcidx_t = nc.dram_tensor("srcidx", [P, max(NIDX, 1)], i32, kind="ExternalInput").ap()
    eaflat_t = nc.dram_tensor("eaflat", [P, max(NIDX, 1) * D], fp16, kind="ExternalInput").ap()
    bigpad_t = nc.dram_tensor("bigpad", [P, cfg["bigpad_cols"]], fp16, kind="ExternalInput").ap()
    invc_t = nc.dram_tensor("invcexp", [P, NT * D], fp16, kind="ExternalInput").ap()
    amp_t = nc.dram_tensor("ampexp", [P, NT * D], fp16, kind="ExternalInput").ap()
    iamp_t = nc.dram_tensor("iampexp", [P, NT * D], fp16, kind="ExternalInput").ap()
    wch_t = nc.dram_tensor("wch", [4, 3, P, 210], fp16, kind="ExternalInput").ap()
    b2r_t = nc.dram_tensor("b2r", [P, 4 * D], fp16, kind="ExternalInput").ap()
    mlpb_t = nc.dram_tensor("mlpb", [P, 53], fp32, kind="ExternalInput").ap()
    w1_t = nc.dram_tensor("w1", [D, 35], fp32, kind="ExternalInput").ap()
    w2_t = nc.dram_tensor("w2", [35, 17], fp32, kind="ExternalInput").ap()
    w3_t = nc.dram_tensor("w3", [17, 1], fp32, kind="ExternalInput").ap()
    poolidx_t = nc.dram_tensor("poolidx", [P, npoolcols], i32, kind="ExternalInput").ap()
    ginvT_t = nc.dram_tensor("ginvT", [P, NGT], fp32, kind="ExternalInput").ap()
    out_g = nc.dram_tensor("out_g", [NGT * P, 1], fp32, kind="ExternalOutput").ap()

    # internal DRAM
    h_own = {l: nc.dram_tensor(f"h_own{l}", [P * NT, D], fp16) for l in range(1, 4)}
    h_own4 = nc.dram_tensor("h_own4", [P * NT + 1, D], fp16)
    hbuf = {l: nc.dram_tensor(f"hbuf{l}", [NPAD + 1, D], fp16, addr_space="Shared") for l in range(1, 4)}

    # persistent SBUF
    idx_sb = nc.alloc_sbuf_tensor("idx_sb", [P, max(NIDX, 1)], i32).ap()
    ea_sb = nc.alloc_sbuf_tensor("ea_sb", [P, max(NIDX, 1) * D], fp16).ap()
    bigpad_sb = nc.alloc_sbuf_tensor("bigpad_sb", [P, cfg["bigpad_cols"]], fp16).ap()
    invc_sb = nc.alloc_sbuf_tensor("invc_sb", [P, NT * D], fp16).ap()
    amp_sb = nc.alloc_sbuf_tensor("amp_sb", [P, NT * D], fp16).ap()
    iamp_sb = nc.alloc_sbuf_tensor("iamp_sb", [P, NT * D], fp16).ap()
    h_sb = [nc.alloc_sbuf_tensor(f"h_sb{k}", [P, NT * D], fp16).ap() for k in range(2)]
    wch_sb = nc.alloc_sbuf_tensor("wch_sb", [P, 4 * 3 * 210], fp16).ap()
    b2_sb = nc.alloc_sbuf_tensor("b2_sb", [P, 4 * D], fp16).ap()
    mlpb_sb = nc.alloc_sbuf_tensor("mlpb_sb", [P, 53], fp32).ap()
    w1_sb = nc.alloc_sbuf_tensor("w1_sb", [D, 35], fp32).ap()
    w2_sb = nc.alloc_sbuf_tensor("w2_sb", [35, 17], fp32).ap()
    w3_sb = nc.alloc_sbuf_tensor("w3_sb", [17, 1], fp32).ap()
    pidx_sb = nc.alloc_sbuf_tensor("pidx_sb", [P, npoolcols], i32).ap()
    ginv_sb = nc.alloc_sbuf_tensor("ginv_sb", [P, NGT], fp32).ap()
    ident16 = nc.alloc_sbuf_tensor("ident16", [P, P], fp16).ap()
    ident32 = nc.alloc_sbuf_tensor("ident32", [P, P], fp32).ap()
    epsb = nc.alloc_sbuf_tensor("epsb", [P, 1], fp32).ap()
    zrow = nc.alloc_sbuf_tensor("zrow", [1, D], fp16).ap()

    cc_sems = {(l, h): nc.alloc_semaphore(name=f"ccs{l}_{h}") for l in range(1, 4) for h in range(2)}

    def do_cc(l, part):
        if part == 0:
            ins_ap = h_own[l].ap()[0:P * NT0, :].opt()
            outs_ap = hbuf[l].ap()[0:NC * P * NT0, :].opt()
        else:
            ins_ap = h_own[l].ap()[P * NT0:P * NT, :].opt()
            outs_ap = hbuf[l].ap()[NC * P * NT0:NC * P * NT, :].opt()
        nc.gpsimd.collective_compute(
            "AllGather", OP.bypass,
            replica_groups=[list(range(NC))],
            ins=[ins_ap], outs=[outs_ap],
        ).then_inc(cc_sems[(l, part)])

    def stage2(pool, spool, scr, psp, a3, s_fin, s2_fin, t0, nb, l, d, hprev, hcur):
        sl = slice(t0 * D, (t0 + nb) * D)
        if d > 0:
            invc3 = invc_sb[:, sl].rearrange("p (t f) -> p t f", t=nb)
            nc.vector.tensor_tensor(out=a3[:, :, 0:D], in0=s_fin, in1=invc3, op=OP.mult)
            u = scr.tile([P, nb * D], fp16, tag="u")
            u3 = u[:].rearrange("p (t f) -> p t f", t=nb)
            nc.vector.tensor_tensor(out=u3, in0=s2_fin, in1=invc3, op=OP.mult)
            v = scr.tile([P, nb * D], fp16, tag="v")
            v3 = v[:].rearrange("p (t f) -> p t f", t=nb)
            nc.vector.tensor_tensor(out=v3, in0=a3[:, :, 0:D], in1=a3[:, :, 0:D], op=OP.mult)
            nc.vector.tensor_tensor(out=u3, in0=u3, in1=v3, op=OP.subtract)
            nc.vector.tensor_scalar_max(u[:], u[:], 0.0)
            nc.scalar.activation(out=a3[:, :, 210:280], in_=u3, func=AF.Sqrt, bias=epsb[:])
        else:
            nc.scalar.activation(out=a3[:, :, 210:280], in_=a3[:, :, 0:D], func=AF.Sqrt, bias=epsb[:])

        sabc = spool.tile([P, nb * 210], fp16, tag="sabc")
        for i in range(nb):
            psmm = psp.tile([P, 210], fp32, space="PSUM", tag="psmm")
            for ch in range(3):
                rows = 128 if ch < 2 else 24
                psT = psp.tile([P, P], fp16, space="PSUM", tag="psT")
                nc.tensor.transpose(out=psT[:rows, :],
                                    in_=a3[:, i:i + 1, ch * 128:ch * 128 + rows].rearrange("p t f -> p (t f)"),
                                    identity=ident16[:])
                aggT = pool.tile([P, P], fp16, tag="aggT")
                nc.vector.tensor_copy(out=aggT[:rows, :], in_=psT[:rows, :])
                nc.tensor.matmul(out=psmm[:, :], lhsT=aggT[:rows, :],
                                 rhs=wch_sb[:rows, (l - 1) * 630 + ch * 210:(l - 1) * 630 + (ch + 1) * 210],
                                 start=(ch == 0), stop=(ch == 2))
            nc.scalar.activation(out=sabc[:, i * 210:(i + 1) * 210], in_=psmm[:, :], func=AF.Copy)

        s3 = sabc[:].rearrange("p (t f) -> p t f", t=nb)
        hn = pool.tile([P, nb * D], fp16, tag="hn")
        hn3 = hn[:].rearrange("p (t f) -> p t f", t=nb)
        tmp = pool.tile([P, nb * D], fp16, tag="tmp")
        tmp3 = tmp[:].rearrange("p (t f) -> p t f", t=nb)
        amp3 = amp_sb[:, sl].rearrange("p (t f) -> p t f", t=nb)
        iamp3 = iamp_sb[:, sl].rearrange("p (t f) -> p t f", t=nb)
        nc.vector.tensor_tensor(out=hn3, in0=s3[:, :, 70:140], in1=amp3, op=OP.mult)
        nc.vector.tensor_tensor(out=tmp3, in0=s3[:, :, 140:210], in1=iamp3, op=OP.mult)
        nc.vector.tensor_tensor(out=hn3, in0=hn3, in1=tmp3, op=OP.add)
        nc.vector.tensor_tensor(out=hn3, in0=hn3, in1=s3[:, :, 0:70], op=OP.add)
        b2b = _insert_axis(b2_sb[:, (l - 1) * D:l * D], 1, nb)
        nc.vector.tensor_tensor(out=hn3, in0=hn3, in1=b2b, op=OP.add)
        nc.scalar.activation(out=hn[:], in_=hn[:], func=AF.Relu)
        nc.vector.tensor_tensor(out=hcur[:, sl], in0=hn[:], in1=hprev[:, sl], op=OP.add)

    def emit_msg_block(blk, l, hprev_full, pool, spool, scr, psp, gpool, hprev, hcur):
        (g, d, t0, nb, col, bcol) = blk
        X = nb * d * D
        is_tail = cfg["has_tail"] and g == cfg["ngroups"] - 1
        gsrc = gpool.tile([P, X], fp16, tag="gsrc")
        for k in range(nb * d):
            cidx = col + k
            nc.gpsimd.indirect_dma_start(
                out=gsrc[:, k * D:(k + 1) * D],
                out_offset=None,
                in_=hprev_full[:, :],
                in_offset=bass.IndirectOffsetOnAxis(ap=idx_sb[:, cidx:cidx + 1], axis=0),
            )
        gblk = spool.tile([P, X], fp16, tag="gblk")
        nc.vector.tensor_tensor(out=gblk[:], in0=gsrc[:], in1=ea_sb[:, col * D:col * D + X], op=OP.add)
        hdst_b = _insert_axis(hprev[:, t0 * D:(t0 + nb) * D].rearrange("p (t f) -> p t f", t=nb), 2, d)
        g3 = gblk[:].rearrange("p (t j f) -> p t j f", t=nb, j=d)
        nc.vector.tensor_tensor(out=g3, in0=g3, in1=hdst_b, op=OP.add)
        nc.scalar.activation(out=gblk[:], in_=gblk[:], func=AF.Relu)

        agg = spool.tile([P, nb * 280], fp16, tag="agg")
        a3 = agg[:].rearrange("p (t f) -> p t f", t=nb)
        mn_out = a3[:, :, 70:140]
        mx_out = a3[:, :, 140:210]
        if d == 1:
            nc.vector.tensor_copy(out=mn_out, in_=gblk[:].rearrange("p (t f) -> p t f", t=nb))
            nc.vector.tensor_copy(out=mx_out, in_=gblk[:].rearrange("p (t f) -> p t f", t=nb))
            nc.vector.tensor_copy(out=a3[:, :, 0:D], in_=gblk[:].rearrange("p (t f) -> p t f", t=nb))
            nc.scalar.activation(out=gblk[:], in_=gblk[:], func=AF.Square)
            s2_fin = gblk[:].rearrange("p (t f) -> p t f", t=nb)
            s_fin = a3[:, :, 0:D]
        else:
            g4 = gblk[:].rearrange("p (t j f) -> p t j f", t=nb, j=d)
            if is_tail:
                mfm = scr.tile([P, X], fp16, tag="mfm")
                m4 = mfm[:].rearrange("p (t j f) -> p t j f", t=nb, j=d)
                bp_b = _insert_axis(bigpad_sb[:, bcol:bcol + nb * d].rearrange("p (t j) -> p t j", t=nb), 3, D)
                nc.vector.tensor_tensor(out=m4, in0=g4, in1=bp_b, op=OP.add)
                _fold_minmax(nc, scr, m4, d, nb, mn_out, OP.min, "mnscr")
            else:
                _fold_minmax(nc, scr, g4, d, nb, mn_out, OP.min, "mnscr")
            _fold_minmax(nc, scr, g4, d, nb, mx_out, OP.max, "mxscr")
            _fold_sum(nc, scr, g4, d, nb, a3[:, :, 0:D], "sscr")
            nc.scalar.activation(out=gblk[:], in_=gblk[:], func=AF.Square)
            s2t = scr.tile([P, nb * D], fp16, tag="s2t")
            _fold_sum(nc, scr, g4, d, nb, s2t[:].rearrange("p (t f) -> p t f", t=nb), "s2scr")
            s2_fin = s2t[:].rearrange("p (t f) -> p t f", t=nb)
            s_fin = a3[:, :, 0:D]
        stage2(pool, spool, scr, psp, a3, s_fin, s2_fin, t0, nb, l, d, hprev, hcur)

    def emit_d0_block(blk, l, pool, spool, scr, psp, hprev, hcur):
        (g, d, t0, nb) = blk
        agg = spool.tile([P, nb * 280], fp16, tag="agg")
        nc.vector.memset(agg[:], 0.0)
        a3 = agg[:].rearrange("p (t f) -> p t f", t=nb)
        stage2(pool, spool, scr, psp, a3, a3[:, :, 0:D], a3[:, :, 0:D], t0, nb, l, 0, hprev, hcur)

    def emit_pooling(pool, spool, psp):
        for t in range(NGT):
            KG = cfg["KG_t"][t]
            pc = cfg["pool_cols"][t]
            pg = pool.tile([P, KG * D], fp16, tag="pg")
            for j in range(KG):
                nc.gpsimd.indirect_dma_start(
                    out=pg[:, j * D:(j + 1) * D], out_offset=None,
                    in_=h_own4.ap()[:, :],
                    in_offset=bass.IndirectOffsetOnAxis(ap=pidx_sb[:, pc + j:pc + j + 1], axis=0))
            gsum = pool.tile([P, D], fp32, tag="gsum")
            nc.vector.tensor_reduce(out=gsum[:], in_=pg[:].rearrange("p (k f) -> p f k", k=KG),
                                    op=OP.add, axis=mybir.AxisListType.X)
            nc.vector.tensor_scalar_mul(gsum[:], gsum[:], ginv_sb[:, t:t + 1])
            psT = psp.tile([P, P], fp32, space="PSUM", tag="psT32")
            nc.tensor.transpose(out=psT[:D, :], in_=gsum[:], identity=ident32[:])
            gT = pool.tile([D, P], fp32, tag="gT")
            nc.vector.tensor_copy(out=gT[:], in_=psT[:D, :])
            ps1 = psp.tile([P, 35], fp32, space="PSUM", tag="psm1")
            nc.tensor.matmul(out=ps1[:], lhsT=gT[:], rhs=w1_sb[:, :], start=True, stop=True)
            y1 = pool.tile([P, 35], fp32, tag="y1")
            nc.vector.tensor_tensor(out=y1[:], in0=ps1[:], in1=mlpb_sb[:, 0:35], op=OP.add)
            nc.scalar.activation(out=y1[:], in_=y1[:], func=AF.Relu)
            psT2 = psp.tile([P, P], fp32, space="PSUM", tag="psT32")
            nc.tensor.transpose(out=psT2[:35, :], in_=y1[:], identity=ident32[:])
            y1T = pool.tile([35, P], fp32, tag="y1T")
            nc.vector.tensor_copy(out=y1T[:], in_=psT2[:35, :])
            ps2 = psp.tile([P, 17], fp32, space="PSUM", tag="psm1")
            nc.tensor.matmul(out=ps2[:], lhsT=y1T[:], rhs=w2_sb[:, :], start=True, stop=True)
            y2 = pool.tile([P, 17], fp32, tag="y2")
            nc.vector.tensor_tensor(out=y2[:], in0=ps2[:], in1=mlpb_sb[:, 35:52], op=OP.add)
            nc.scalar.activation(out=y2[:], in_=y2[:], func=AF.Relu)
            psT3 = psp.tile([P, P], fp32, space="PSUM", tag="psT32")
            nc.tensor.transpose(out=psT3[:17, :], in_=y2[:], identity=ident32[:])
            y2T = pool.tile([17, P], fp32, tag="y2T")
            nc.vector.tensor_copy(out=y2T[:], in_=psT3[:17, :])
            ps3 = psp.tile([P, 1], fp32, space="PSUM", tag="psm1")
            nc.tensor.matmul(out=ps3[:], lhsT=y2T[:], rhs=w3_sb[:, :], start=True, stop=True)
            y3 = pool.tile([P, 1], fp32, tag="y3")
            nc.vector.tensor_tensor(out=y3[:], in0=ps3[:], in1=mlpb_sb[:, 52:53], op=OP.add)
            nc.sync.dma_start(out=out_g[t * P:(t + 1) * P, :], in_=y3[:])

    msgA = [b for b in cfg["msg_blocks"] if b[2] < NT0]
    msgB = [b for b in cfg["msg_blocks"] if b[2] >= NT0]
    d0A = [b for b in cfg["blocks"] if b[1] == 0 and b[2] < NT0]
    d0B = [b for b in cfg["blocks"] if b[1] == 0 and b[2] >= NT0]
    for b in msgA:
        assert b[2] + b[3] <= NT0, b

    for l in range(1, 5):
        hprev_full = h0_full_t if l == 1 else hbuf[l - 1].ap()
        hprev = h_sb[(l - 1) % 2]
        hcur = h_sb[l % 2]
        if l >= 2:
            nc.gpsimd.wait_ge(cc_sems[(l - 1, 0)], 1)
            nc.gpsimd.wait_ge(cc_sems[(l - 1, 1)], 1)
        if l < 4:
            with tile.TileContext(nc) as tc:
                with tc.tile_pool(name=f"L{l}a", bufs=2) as pool, \
                     tc.tile_pool(name=f"Lg{l}a", bufs=3) as gpool, \
                     tc.tile_pool(name=f"Ls{l}a", bufs=2) as spool, \
                     tc.tile_pool(name=f"Lc{l}a", bufs=1) as scr, \
                     tc.tile_pool(name=f"Lp{l}a", bufs=2, space="PSUM") as psp:
                    if l == 1:
                        nc.sync.dma_start(out=idx_sb[:, :], in_=srcidx_t[:, :])
                        nq = 4
                        step = (max(NIDX, 1) * D + nq - 1) // nq
                        for q in range(nq):
                            s0, s1 = q * step, min((q + 1) * step, max(NIDX, 1) * D)
                            if s0 < s1:
                                nc.sync.dma_start(out=ea_sb[:, s0:s1], in_=eaflat_t[:, s0:s1])
                        nc.sync.dma_start(out=h_sb[0][:, :], in_=h0_own_t[:, :])
                        nc.sync.dma_start(out=bigpad_sb[:, :], in_=bigpad_t[:, :])
                        nc.sync.dma_start(out=invc_sb[:, :], in_=invc_t[:, :])
                        nc.sync.dma_start(out=amp_sb[:, :], in_=amp_t[:, :])
                        nc.sync.dma_start(out=iamp_sb[:, :], in_=iamp_t[:, :])
                        nc.sync.dma_start(out=wch_sb[:].rearrange("p (l c f) -> p l c f", l=4, c=3),
                                          in_=wch_t.rearrange("l c p f -> p l c f"))
                        nc.sync.dma_start(out=b2_sb[:, :], in_=b2r_t[:, :])
                        nc.sync.dma_start(out=mlpb_sb[:, :], in_=mlpb_t[:, :])
                        nc.sync.dma_start(out=w1_sb[:, :], in_=w1_t[:, :])
                        nc.sync.dma_start(out=w2_sb[:, :], in_=w2_t[:, :])
                        nc.sync.dma_start(out=w3_sb[:, :], in_=w3_t[:, :])
                        nc.sync.dma_start(out=pidx_sb[:, :], in_=poolidx_t[:, :])
                        nc.sync.dma_start(out=ginv_sb[:, :], in_=ginvT_t[:, :])
                        make_identity(nc, ident16[:])
                        make_identity(nc, ident32[:])
                        nc.vector.memset(epsb[:], STD_EPS)
                        nc.vector.memset(zrow[:], 0.0)
                        for ll in range(1, 4):
                            nc.sync.dma_start(out=hbuf[ll].ap()[NPAD:NPAD + 1, :], in_=zrow[:])
                        nc.sync.dma_start(out=h_own4.ap()[P * NT:P * NT + 1, :], in_=zrow[:])
                    for blk in msgA:
                        emit_msg_block(blk, l, hprev_full, pool, spool, scr, psp, gpool, hprev, hcur)
                    for blk in d0A:
                        emit_d0_block(blk, l, pool, spool, scr, psp, hprev, hcur)
                    nc.sync.dma_start(
                        out=h_own[l].ap()[0:P * NT0, :].rearrange("(p t) f -> p (t f)", p=P),
                        in_=hcur[:, 0:NT0 * D])
            do_cc(l, 0)
            with tile.TileContext(nc) as tc:
                with tc.tile_pool(name=f"L{l}b", bufs=2) as pool, \
                     tc.tile_pool(name=f"Lg{l}b", bufs=3) as gpool, \
                     tc.tile_pool(name=f"Ls{l}b", bufs=2) as spool, \
                     tc.tile_pool(name=f"Lc{l}b", bufs=1) as scr, \
                     tc.tile_pool(name=f"Lp{l}b", bufs=2, space="PSUM") as psp:
                    for blk in msgB:
                        emit_msg_block(blk, l, hprev_full, pool, spool, scr, psp, gpool, hprev, hcur)
                    for blk in d0B:
                        emit_d0_block(blk, l, pool, spool, scr, psp, hprev, hcur)
                    nc.sync.dma_start(
                        out=h_own[l].ap()[P * NT0:P * NT, :].rearrange("(p t) f -> p (t f)", p=P),
                        in_=hcur[:, NT0 * D:NT * D])
            do_cc(l, 1)
        else:
            with tile.TileContext(nc) as tc:
                with tc.tile_pool(name=f"L{l}", bufs=2) as pool, \
                     tc.tile_pool(name=f"Lg{l}", bufs=3) as gpool, \
                     tc.tile_pool(name=f"Ls{l}", bufs=2) as spool, \
                     tc.tile_pool(name=f"Lc{l}", bufs=1) as scr, \
                     tc.tile_pool(name=f"Lp{l}", bufs=2, space="PSUM") as psp:
                    for blk in msgA + msgB:
                        emit_msg_block(blk, l, hprev_full, pool, spool, scr, psp, gpool, hprev, hcur)
                    for blk in d0A + d0B:
                        emit_d0_block(blk, l, pool, spool, scr, psp, hprev, hcur)
                    nc.sync.dma_start(
                        out=h_own4.ap()[0:P * NT, :].rearrange("(p t) f -> p (t f)", p=P),
                        in_=hcur[:, :])
                    emit_pooling(pool, spool, psp)

    nc.compile()
    return nc


def kernel(x, edge_index, edge_attr, batch, atom_emb, post_w, post_b,
           bn_gamma, bn_beta, mlp_w1, mlp_b1, mlp_w2, mlp_b2, mlp_w3, mlp_b3):
    cfg, arrays, asm = _prep(x, edge_index, edge_attr, batch, atom_emb)
    wd = _prep_weights(post_w, post_b, bn_gamma, bn_beta, mlp_w1, mlp_b1,
                       mlp_w2, mlp_b2, mlp_w3, mlp_b3)
    nc = _build(cfg)

    in_maps = []
    for c in range(NC):
        in_maps.append({
            "h0_own": arrays["h0_own"][c],
            "h0_full": arrays["h0_full"],
            "srcidx": arrays["srcidx"][c],
            "eaflat": arrays["eaflat"][c].reshape(P, -1),
            "bigpad": arrays["bigpad"][c],
            "invcexp": arrays["invcexp"][c],
            "ampexp": arrays["ampexp"][c],
            "iampexp": arrays["iampexp"][c],
            "wch": wd["wch"],
            "b2r": wd["b2r"],
            "mlpb": wd["mlpb"],
            "w1": wd["w1"],
            "w2": wd["w2"],
            "w3": wd["w3"],
            "poolidx": arrays["poolidx"][c],
            "ginvT": arrays["ginvT"][c],
        })
    import os
    trace = os.environ.get("KERNEL_TRACE", "0") == "1"
    res = run_bass_kernel_spmd(nc, in_maps, core_ids=list(range(NC)), trace=trace)
    kernel.last_exec_time_ns = res.exec_time_ns
    y = np.zeros((G, 1), np.float32)
    for c in range(NC):
        og = res.results[c]["out_g"]
        cg = asm["core_graphs"][c]
        y[cg] = og[:len(cg)]
    return y
